# revision 7
# baseline (speedup 1.0000x reference)
"""Trainium2 Bass kernel for a 3-layer GNN message-passing block.

Reference computation (per layer i):
    x1 = h @ Wfc[i] + bfc[i]                        # [N_SUB, D]
    x2 = scatter_mean(h, idx) @ Wsum[i] + bsum[i]   # [NUM_GRAPHS, D]
    h  = elu(x1 + x2[idx])
then
    out = relu(scatter_mean(h, idx) @ Wf1 + bf1) @ Wf2 + bf2

Strategy: data-parallel over 8 NeuronCores (sorted graph index -> contiguous
graph windows per core; scatter/gather as one-hot matmuls on TensorE). The
kernel runs "feature-major": h is held transposed (hT[d, row]) so each fc
layer is W-stationary and needs no on-chip transposition; the ELU writes the
next layer's hT directly. Layer-1 fc runs in fp16; layers 2/3 fc and all
scatters run as fp8 DoubleRow matmuls (2 fp8 MACs per PE cell per cycle,
halving TensorE time on those ops). Precision holds because (a) the fp8
weight-quantization residual dW = Wfc - q8(Wfc) is folded into Wsum host-side
(x1 and x2 add into the same pre-activation, and h rows share a large
per-graph common mode that mean @ dW recovers), and (b) layer-1 and the
gather/x2/head paths stay fp16. Scatter inputs (row-major h) come from 2-byte
DMA transposes of fp8 row PAIRS (bitcast to u16), whose byte-interleaved
output is legal for the DoubleRow moving operand.
"""

import numpy as np

P = 128
D = 512
N_SUB = 100000
NUM_GRAPHS = 4096
N_LAYERS = 3
NUM_TASKS = 10
N_CORES = 8
WIN_PER_CORE = 5
CH_PER_WIN = 20
ROWS_PER_WIN = CH_PER_WIN * P            # 2560
N_LOC = WIN_PER_CORE * ROWS_PER_WIN      # 12800 padded rows per core
CHUNKS = N_LOC // P                      # 100
G_WIN = P                                # graph slots per window
G_LOC = WIN_PER_CORE * G_WIN             # 640 graph slots per core
N_WIN_TOTAL = N_CORES * WIN_PER_CORE     # 40
DBLK = D // P                            # 4
D2 = 2 * D                               # 1024
D2BLK = D2 // P
RG_ROWS = 4 * P                          # 512 rows per rowgroup
N_RG = N_LOC // RG_ROWS                  # 25 rowgroups per core
RG_WIN = ROWS_PER_WIN // RG_ROWS         # 5 rowgroups per window
N_HRG = 2 * N_RG                         # 50 half-rowgroups per core

_cached = {}


def _f16():
    return np.float16


def _f8():
    import ml_dtypes
    return ml_dtypes.float8_e4m3fn


def _q8(x):
    f8 = _f8()
    return np.asarray(x, np.float32).astype(f8).astype(np.float32)


# ----------------------------------------------------------------- host prep

def _pack_windows(counts):
    """Split graphs 0..NUM_GRAPHS-1 into N_WIN_TOTAL contiguous windows with
    <= G_WIN graphs and <= ROWS_PER_WIN rows each, roughly row-balanced."""
    total = int(counts.sum())
    target = total / N_WIN_TOTAL
    wins = []
    g = 0
    rows_done = 0
    for w in range(N_WIN_TOTAL):
        g0 = g
        rows_w = 0
        while g < NUM_GRAPHS:
            c = int(counts[g])
            if rows_w + c > ROWS_PER_WIN or (g - g0) >= G_WIN:
                break
            if (w < N_WIN_TOTAL - 1 and rows_w > 0
                    and rows_done + rows_w + c > (w + 1) * target):
                remaining = total - (rows_done + rows_w)
                if remaining <= (N_WIN_TOTAL - w - 1) * ROWS_PER_WIN * 0.98:
                    break
            rows_w += c
            g += 1
        while g < NUM_GRAPHS and counts[g] == 0 and (g - g0) < G_WIN:
            g += 1
        rows_done += rows_w
        wins.append((g0, g))
    assert g == NUM_GRAPHS, f"window packing failed: {g}/{NUM_GRAPHS}"
    return wins


def _build_core_inputs(h, idx, counts, starts, wins, core, shared):
    f16 = _f16()
    f8 = _f8()
    h_pad = np.zeros((N_LOC, D), dtype=np.float32)
    slot = np.full(N_LOC, -1, dtype=np.int64)
    invc = np.zeros((P, WIN_PER_CORE), dtype=np.float32)  # [g_in_win, w]
    gmap = []
    for lw in range(WIN_PER_CORE):
        g0, g1 = wins[core * WIN_PER_CORE + lw]
        r0, r1 = int(starts[g0]), int(starts[g1])
        n = r1 - r0
        h_pad[lw * ROWS_PER_WIN: lw * ROWS_PER_WIN + n] = h[r0:r1]
        slot[lw * ROWS_PER_WIN: lw * ROWS_PER_WIN + n] = \
            lw * G_WIN + (idx[r0:r1] - g0)
        for j, g in enumerate(range(g0, g1)):
            invc[j, lw] = 1.0 / max(int(counts[g]), 1)
            gmap.append((g, lw * G_WIN + j))
    # gather one-hot, per chunk c: wga[c][g, r] = 1 iff slot(c*128+r) = g (rel)
    wga = np.zeros((CHUNKS, P, P), dtype=np.float32)
    for c in range(CHUNKS):
        w = c // CH_PER_WIN
        s = slot[c * P:(c + 1) * P]
        real = np.nonzero(s >= 0)[0]
        wga[c][s[real] - w * G_WIN, real] = 1.0
    wga_flat = np.ascontiguousarray(
        np.transpose(wga, (1, 0, 2)).reshape(P, CHUNKS * P)).astype(f16)
    # scatter one-hot DoubleRow pairs, per half-rowgroup hr:
    # wsc8[p, hr, i, g] = 1 iff slot(hr*256 + 2p + i) = g (window-relative)
    wsc8 = np.zeros((P, N_HRG, 2, P), dtype=np.float32)
    for hr in range(N_HRG):
        w = hr // (2 * RG_WIN)
        s = slot[hr * 2 * P:(hr + 1) * 2 * P].reshape(P, 2)
        for i in range(2):
            real = np.nonzero(s[:, i] >= 0)[0]
            wsc8[real, hr, i, s[real, i] - w * G_WIN] = 1.0
    wsc8_flat = np.ascontiguousarray(wsc8.reshape(P, N_HRG * 2 * P)).astype(f8)
    # transposed h0, rowgroup-major: h16t[p, rg, b, r] = h[rg*512+r, 128b+p]
    h3 = h_pad.reshape(N_RG, RG_ROWS, DBLK, P)
    h16t = np.ascontiguousarray(
        h3.transpose(3, 0, 2, 1).reshape(P, N_RG * DBLK * RG_ROWS)).astype(f16)
    # layer-0 scatter_mean + x2 computed host-side (depends only on inputs)
    Wsum0, cbias0 = shared["_wsum0"], shared["_cbias0"]
    ssum = np.zeros((G_LOC, D), dtype=np.float32)
    np.add.at(ssum, slot[slot >= 0], h_pad[slot >= 0])
    mean0 = ssum * invc.T.reshape(G_LOC, 1)
    x2w0 = (mean0 @ Wsum0 + cbias0).astype(f16)      # [G_LOC, D]
    x2w0b = np.ascontiguousarray(
        x2w0.reshape(WIN_PER_CORE, G_WIN, D).transpose(1, 0, 2)
        .reshape(G_WIN, WIN_PER_CORE * D))
    in_map = {
        "h16t": h16t,
        "x2w0b": x2w0b,
        "wsc8": wsc8_flat,
        "wga": wga_flat,
        "invc": invc,
        **{k: v for k, v in shared.items() if not k.startswith("_")},
    }
    return in_map, gmap


def _prep_shared(Wfc, bfc, Wsum, bsum, Wf1, bf1, Wf2, bf2):
    f16 = _f16()
    f8 = _f8()
    # layer-0 fc weights fp16 (stationary):
    # wfc0T[p, db, dblk, m] = Wfc[0][128db + p, 128dblk + m]
    wfc0 = np.ascontiguousarray(
        Wfc[0].reshape(DBLK, P, DBLK, P).transpose(1, 0, 2, 3)
        .reshape(P, DBLK * DBLK * P)).astype(f16)
    # layers 1/2 fc weights fp8 DoubleRow pairs (stationary):
    # wfc8[p, li, dblk, b, i, m] = Wfc[l][128(2b+i) + p, 128dblk + m]
    wfc8 = np.zeros((P, 2, DBLK, 2, 2, P), dtype=np.float32)
    for li, l in enumerate((1, 2)):
        Wl = Wfc[l].astype(np.float32)
        for dblk in range(DBLK):
            for b in range(2):
                for i in range(2):
                    wfc8[:, li, dblk, b, i, :] = \
                        Wl[128 * (2 * b + i) + np.arange(P)][:, 128 * dblk:128 * (dblk + 1)]
    wfc8 = np.ascontiguousarray(wfc8.reshape(P, 2 * DBLK * 2 * 2 * P)).astype(f8)
    # fold fp8 weight-quantization residual into Wsum for layers 1/2
    Wsum_eff = Wsum.astype(np.float32).copy()
    for l in (1, 2):
        Wsum_eff[l] += Wfc[l].astype(np.float32) - _q8(Wfc[l])
    wsum = np.ascontiguousarray(
        Wsum_eff.reshape(N_LAYERS, DBLK, P, D).transpose(2, 0, 1, 3)
        .reshape(P, N_LAYERS * DBLK * D)).astype(f16)
    cbias = np.ascontiguousarray((bfc + bsum).reshape(1, N_LAYERS * D)).astype(f16)
    # wf1 rhs blocks [b][half] = Wf1[b*P:(b+1)*P, half*D:(half+1)*D]
    wf1 = np.ascontiguousarray(
        Wf1.reshape(DBLK, P, 2, D).transpose(1, 0, 2, 3)
        .reshape(P, DBLK * D2)).astype(f16)
    bf1w = np.ascontiguousarray(bf1.reshape(1, D2)).astype(f16)
    # wf2 blocks [q] = Wf2[q*P:(q+1)*P, :]; packed [P, 8*NUM_TASKS]
    wf2 = np.ascontiguousarray(
        Wf2.reshape(D2BLK, P, NUM_TASKS).transpose(1, 0, 2)
        .reshape(P, D2BLK * NUM_TASKS)).astype(f16)
    bf2w = np.ascontiguousarray(bf2.reshape(1, NUM_TASKS)).astype(f16)
    return {
        "wfc0": wfc0, "wfc8": wfc8, "wsum": wsum, "cbias": cbias,
        "wf1": wf1, "bf1w": bf1w, "wf2": wf2, "bf2w": bf2w,
        "_wsum0": Wsum[0].astype(np.float32),
        "_cbias0": (bfc[0] + bsum[0]).astype(np.float32)[None, :],
    }


# -------------------------------------------------------------- bass program

def _build_program():
    from contextlib import ExitStack

    import concourse.mybir as mybir
    import concourse.tile as tile
    from concourse import bacc

    bf = mybir.dt.float16
    f8 = mybir.dt.float8e4
    f32 = mybir.dt.float32
    AF = mybir.ActivationFunctionType
    ALU = mybir.AluOpType
    DR = mybir.MatmulPerfMode.DoubleRow

    nc = bacc.Bacc("TRN2", debug=False, target_bir_lowering=False,
                   num_devices=N_CORES, dynamic_dma_scratch_size=2048)

    RGB = DBLK * RG_ROWS                 # 2048: elems per rowgroup per partition

    h16t_d = nc.dram_tensor("h16t", [P, N_RG * RGB], bf, kind="ExternalInput")
    x2w0_d = nc.dram_tensor("x2w0b", [G_WIN, WIN_PER_CORE * D], bf,
                            kind="ExternalInput")
    wsc8_d = nc.dram_tensor("wsc8", [P, N_HRG * 2 * P], f8, kind="ExternalInput")
    wga_d = nc.dram_tensor("wga", [P, CHUNKS * P], bf, kind="ExternalInput")
    invc_d = nc.dram_tensor("invc", [P, WIN_PER_CORE], f32, kind="ExternalInput")
    wfc0_d = nc.dram_tensor("wfc0", [P, DBLK * DBLK * P], bf, kind="ExternalInput")
    wfc8_d = nc.dram_tensor("wfc8", [P, 2 * DBLK * 2 * 2 * P], f8,
                            kind="ExternalInput")
    wsum_d = nc.dram_tensor("wsum", [P, N_LAYERS * DBLK * D], bf, kind="ExternalInput")
    cbias_d = nc.dram_tensor("cbias", [1, N_LAYERS * D], bf, kind="ExternalInput")
    wf1_d = nc.dram_tensor("wf1", [P, DBLK * D2BLK * P], bf, kind="ExternalInput")
    bf1_d = nc.dram_tensor("bf1w", [1, D2], bf, kind="ExternalInput")
    wf2_d = nc.dram_tensor("wf2", [P, D2BLK * NUM_TASKS], bf, kind="ExternalInput")
    bf2_d = nc.dram_tensor("bf2w", [1, NUM_TASKS], bf, kind="ExternalInput")
    out_d = nc.dram_tensor("out", [NUM_TASKS, G_LOC], f32, kind="ExternalOutput")

    with tile.TileContext(nc) as tc, ExitStack() as ctx:
        const = ctx.enter_context(tc.tile_pool(name="const", bufs=1))
        hpool = ctx.enter_context(tc.tile_pool(name="h", bufs=1))
        stream = ctx.enter_context(tc.tile_pool(name="stream", bufs=4))
        work = ctx.enter_context(tc.tile_pool(name="work", bufs=2))
        x2pool = ctx.enter_context(tc.tile_pool(name="x2", bufs=1))
        psum = ctx.enter_context(tc.tile_pool(name="psum", bufs=2, space="PSUM"))
        psx1 = ctx.enter_context(tc.tile_pool(name="psx1", bufs=4, space="PSUM"))

        # ---- constants
        ones = const.tile([1, P], bf, tag="ones")
        nc.vector.memset(ones[:], 1.0)
        x2w0_t = []
        for w in range(WIN_PER_CORE):
            t0w = x2pool.tile([P, D], bf, tag=f"x2w{w}", name=f"x2w0_{w}")
            nc.sync.dma_start(t0w[:], x2w0_d[:, w * D:(w + 1) * D])
            x2w0_t.append(t0w)
        wfc0_t = const.tile([P, DBLK * DBLK * P], bf, tag="wfc0")
        nc.sync.dma_start(wfc0_t[:], wfc0_d[:, :])
        invc_t = const.tile([P, WIN_PER_CORE], f32, tag="invc")
        nc.sync.dma_start(invc_t[:], invc_d[:, :])
        # h0T rowgroup stream (fp16 from host, ring)
        h0_ring = []

        def load_h0(rg):
            t = stream.tile([P, RGB], bf, tag="h0", name=f"h0_{rg}", bufs=3)
            nc.sync.dma_start(t[:], h16t_d[:, rg * RGB:(rg + 1) * RGB])
            h0_ring.append(t)

        for rg in range(3):
            load_h0(rg)
        WCH = CH_PER_WIN * P
        wgab_w = [const.tile([P, WCH], bf, tag=f"wgab{k}", name=f"wgab{k}")
                  for k in range(WIN_PER_CORE)]
        wscb_w = [const.tile([P, 2 * RG_WIN * 2 * P], f8, tag=f"wscb{k}",
                             name=f"wscb{k}")
                  for k in range(WIN_PER_CORE)]
        nc.sync.dma_start(wgab_w[0][:], wga_d[:, 0 * WCH:1 * WCH])
        WSC_W = 2 * RG_WIN * 2 * P
        for k in range(WIN_PER_CORE):
            nc.sync.dma_start(wscb_w[k][:], wsc8_d[:, k * WSC_W:(k + 1) * WSC_W])
            if k + 1 < WIN_PER_CORE:
                nc.sync.dma_start(wgab_w[k + 1][:],
                                  wga_d[:, (k + 1) * WCH:(k + 2) * WCH])
            load_h0(3 + k)
        for rg in range(8, N_RG):
            load_h0(rg)
        # remaining constants on the ACT HWDGE queue
        wfc8_t = const.tile([P, 2 * DBLK * 2 * 2 * P], f8, tag="wfc8")
        nc.scalar.dma_start(wfc8_t[:], wfc8_d[:, :])
        wsumb = const.tile([P, N_LAYERS * DBLK * D], bf, tag="wsumb")
        nc.scalar.dma_start(wsumb[:], wsum_d[:, :])
        cbiasb = const.tile([1, N_LAYERS * D], bf, tag="cbiasb")
        nc.scalar.dma_start(cbiasb[:], cbias_d[:, :])
        wf1b = const.tile([P, DBLK * D2BLK * P], bf, tag="wf1b")
        nc.scalar.dma_start(wf1b[:], wf1_d[:, :])
        bf1_t = const.tile([1, D2], bf, tag="bf1")
        nc.scalar.dma_start(bf1_t[:], bf1_d[:, :])
        wf2b = const.tile([P, D2BLK * NUM_TASKS], bf, tag="wf2b")
        nc.scalar.dma_start(wf2b[:], wf2_d[:, :])
        bf2_t = const.tile([1, NUM_TASKS], bf, tag="bf2")
        nc.scalar.dma_start(bf2_t[:], bf2_d[:, :])

        # hT generations: A written by layers 0 and 2, B by layer 1
        hA = [hpool.tile([P, RGB], f8, tag=f"hA{rg}", name=f"hA{rg}")
              for rg in range(N_RG)]
        hB = [hpool.tile([P, RGB], f8, tag=f"hB{rg}", name=f"hB{rg}")
              for rg in range(N_RG)]

        def wfc0_s(db, dblk):
            return wfc0_t[:, (db * DBLK + dblk) * P:(db * DBLK + dblk + 1) * P]

        def wfc8_s(layer, dblk, b):
            li = layer - 1
            off = ((li * DBLK + dblk) * 2 + b) * 2 * P
            return wfc8_t[:, off:off + 2 * P].rearrange("p (i m) -> p i m", i=2)

        def wsum_s(layer, b):
            return wsumb[:, (layer * DBLK + b) * D:(layer * DBLK + b + 1) * D]

        def cbias_s(layer):
            return cbiasb[:, layer * D:(layer + 1) * D]

        def wf1_h(b, half):
            i = b * 2 + half
            return wf1b[:, i * D:(i + 1) * D]

        def wf2_s(q):
            return wf2b[:, q * NUM_TASKS:(q + 1) * NUM_TASKS]

        def wga_rg(rg):
            k = rg // RG_WIN
            j = rg % RG_WIN
            return wgab_w[k][:, j * RG_ROWS:(j + 1) * RG_ROWS]

        def wsc_hr(hr):
            k = hr // (2 * RG_WIN)
            j = hr % (2 * RG_WIN)
            return wscb_w[k][:, j * 2 * P:(j + 1) * 2 * P].rearrange(
                "p (i g) -> p i g", i=2)

        def x2_window(meanT, w, layer):
            """x2 = meanT.T @ Wsum_eff + (bfc+bsum), as fp16 [g, d]."""
            ps = psum.tile([P, D], f32, tag="x2")
            for b in range(DBLK):
                nc.tensor.matmul(ps[:], lhsT=meanT[:, b * P:(b + 1) * P],
                                 rhs=wsum_s(layer, b),
                                 start=(b == 0), stop=False)
            nc.tensor.matmul(ps[:], lhsT=ones[:, :P], rhs=cbias_s(layer),
                             start=False, stop=True)
            x2w = x2pool.tile([P, D], bf, tag=f"x2w{w}", name=f"x2w{layer}_{w}")
            nc.scalar.activation(x2w[:], ps[:], AF.Copy)
            return x2w[:]

        x2ws = {w: x2w0_t[w][:] for w in range(WIN_PER_CORE)}

        out_sb = const.tile([NUM_TASKS, G_LOC], f32, tag="out")

        head_pend = {}

        def head_t(w, meanT):
            """t = relu(hg @ Wf1 + bf1), transposed; out-MMs deferred."""
            t = work.tile([P, D2], bf, tag="tT", bufs=1, name=f"t{w}")
            tTh = []
            for half in range(2):
                ps = psx1.tile([P, D], f32, tag="x1", name=f"hps{w}_{half}")
                for b in range(DBLK):
                    nc.tensor.matmul(ps[:], lhsT=meanT[:, b * P:(b + 1) * P],
                                     rhs=wf1_h(b, half),
                                     start=(b == 0), stop=False)
                nc.tensor.matmul(ps[:], lhsT=ones[:, :P],
                                 rhs=bf1_t[:, half * D:(half + 1) * D],
                                 start=False, stop=True)
                nc.scalar.activation(t[:, half * D:(half + 1) * D],
                                     ps[:], AF.Relu)
                th = work.tile([P, D], bf, tag=f"tTh{half}", bufs=2,
                               name=f"tTh{w}_{half}")
                nc.scalar.dma_start(th[:].rearrange("p (b r) -> p b r", b=DBLK),
                                    t[:, half * D:(half + 1) * D],
                                    transpose=True)
                tTh.append(th)
            head_pend[w] = tTh

        def head_out(w):
            tTh = head_pend.pop(w)
            pso = psum.tile([NUM_TASKS, P], f32, tag="x2", name=f"pso{w}")
            for q in range(D2BLK):
                nc.tensor.matmul(pso[:], lhsT=wf2_s(q),
                                 rhs=tTh[q // 4][:, (q % 4) * P:
                                                 (q % 4 + 1) * P],
                                 start=(q == 0), stop=False)
            nc.tensor.matmul(pso[:], lhsT=bf2_t[:], rhs=ones[:, :P],
                             start=False, stop=True)
            nc.vector.tensor_copy(out_sb[:, w * P:(w + 1) * P], pso[:])

        # ---- layer passes; each folds the NEXT context's scatter (layer
        # l+1's, or the head's) in with a small lag so the PE never has a
        # serial scatter phase.
        LAG_RG = 1      # rowgroups between ELU completion and its scatter
        for layer in range(N_LAYERS):
            h_in = hA if layer == 1 else hB      # layer 0 reads h0_ring
            h_out = hB if layer == 1 else hA
            if layer == 0:
                h_in = None
            nxt_x2ws = {}
            sc_state = {}
            pend = {}

            def emit_rg_scatter(rg, layer=layer, h_out=h_out,
                                sc_state=sc_state, pend=pend):
                """u16-transpose rowgroup rg of h_out, 2 DoubleRow scatter MMs."""
                iw = rg % RG_WIN
                w = rg // RG_WIN
                hrow = []
                for B in range(2):
                    hr_t = work.tile([P, 2 * D], f8, tag="hrow", bufs=4,
                                     name=f"hrow{layer}_{rg}_{B}")
                    for b in range(DBLK):
                        nc.scalar.dma_start(
                            hr_t[:, 2 * P * b:2 * P * (b + 1)].bitcast(bf),
                            h_out[rg][:, b * RG_ROWS + 2 * P * B:
                                      b * RG_ROWS + 2 * P * (B + 1)].bitcast(bf),
                            transpose=True)
                    hrow.append(hr_t)
                if iw == 0:
                    sc_state["ps"] = psum.tile([P, D], f32, tag="sc",
                                               name=f"sc{layer}_{w}")
                for B in range(2):
                    nc.tensor.matmul(
                        sc_state["ps"][:], lhsT=wsc_hr(rg * 2 + B),
                        rhs=hrow[B][:].rearrange("p (d i) -> p i d", i=2),
                        start=(iw == 0 and B == 0),
                        stop=(iw == RG_WIN - 1 and B == 1),
                        perf_mode=DR)
                if iw == RG_WIN - 1:
                    mean = work.tile([P, D], bf, tag="mean", bufs=1,
                                     name=f"mean{layer}_{w}")
                    nc.scalar.activation(mean[:], sc_state["ps"][:], AF.Copy,
                                         scale=invc_t[:, w:w + 1])
                    meanT = work.tile([P, D], bf, tag="meanT", bufs=1,
                                      name=f"meanT{layer}_{w}")
                    nc.scalar.dma_start(
                        meanT[:].rearrange("p (b r) -> p b r", b=DBLK),
                        mean[:], transpose=True)
                    pend[w] = meanT

            def emit_window_tail(w, layer=layer, nxt_x2ws=nxt_x2ws,
                                 pend=pend):
                meanT = pend.pop(w)
                if layer < N_LAYERS - 1:
                    nxt_x2ws[w] = x2_window(meanT, w, layer + 1)
                else:
                    head_t(w, meanT)

            for rg in range(N_RG):
                w = rg // RG_WIN
                for dblk in range(DBLK):
                    ps = psx1.tile([P, RG_ROWS], f32, tag="x1")
                    if layer == 0:
                        for db in range(DBLK):
                            nc.tensor.matmul(
                                ps[:], lhsT=wfc0_s(db, dblk),
                                rhs=h0_ring[rg][:, db * RG_ROWS:
                                                (db + 1) * RG_ROWS],
                                start=(db == 0), stop=False)
                    else:
                        for b in range(2):
                            rhs = h_in[rg][:, 2 * b * RG_ROWS:
                                           (2 * b + 2) * RG_ROWS].rearrange(
                                "p (i r) -> p i r", i=2)
                            nc.tensor.matmul(ps[:], lhsT=wfc8_s(layer, dblk, b),
                                             rhs=rhs, start=(b == 0),
                                             stop=False, perf_mode=DR)
                    nc.tensor.matmul(
                        ps[:], lhsT=x2ws[w][:, dblk * P:(dblk + 1) * P],
                        rhs=wga_rg(rg), start=False, stop=True)
                    # ELU: h = max(z, min(exp(z), 1) - 1)
                    e = work.tile([P, RG_ROWS], bf, tag="e")
                    nc.scalar.activation(e[:], ps[:], AF.Exp)
                    me = work.tile([P, RG_ROWS], bf, tag="me")
                    nc.vector.tensor_scalar(me[:], e[:], 1.0, -1.0,
                                            op0=ALU.min, op1=ALU.add)
                    nc.vector.tensor_tensor(
                        h_out[rg][:, dblk * RG_ROWS:(dblk + 1) * RG_ROWS],
                        ps[:], me[:], op=ALU.max)
                if layer == 0 and rg + 3 < N_RG:
                    pass  # h0 loads all queued up front; ring paces them
                if rg >= LAG_RG:
                    emit_rg_scatter(rg - LAG_RG)
                wt = (rg - LAG_RG - 2) // RG_WIN
                if (rg >= LAG_RG + 2
                        and (rg - LAG_RG - 2) % RG_WIN == RG_WIN - 1):
                    emit_window_tail(wt)
                wt2 = (rg - LAG_RG - 4) // RG_WIN
                if (rg >= LAG_RG + 4
                        and (rg - LAG_RG - 4) % RG_WIN == RG_WIN - 1
                        and wt2 in head_pend):
                    head_out(wt2)
            for rg in range(N_RG - LAG_RG, N_RG):
                emit_rg_scatter(rg)
            for w in sorted(pend):
                emit_window_tail(w)
            for w in sorted(head_pend):
                head_out(w)
            x2ws = nxt_x2ws

        nc.sync.dma_start(out_d[:, :], out_sb[:])

    nc.compile()
    return nc


# ------------------------------------------------------------------- kernel

def kernel(**inputs):
    h = np.asarray(inputs["h_subgraph"], dtype=np.float32)
    idx = np.asarray(inputs["subgraph_idx_batch"]).astype(np.int64)
    if not np.all(idx[:-1] <= idx[1:]):        # defensive: index must be sorted
        order = np.argsort(idx, kind="stable")
        h, idx = h[order], idx[order]

    counts = np.bincount(idx, minlength=NUM_GRAPHS)
    starts = np.concatenate([[0], np.cumsum(counts)])
    wins = _pack_windows(counts)
    shared = _prep_shared(
        np.asarray(inputs["Wfc"], np.float32), np.asarray(inputs["bfc"], np.float32),
        np.asarray(inputs["Wsum"], np.float32), np.asarray(inputs["bsum"], np.float32),
        np.asarray(inputs["Wf1"], np.float32), np.asarray(inputs["bf1"], np.float32),
        np.asarray(inputs["Wf2"], np.float32), np.asarray(inputs["bf2"], np.float32),
    )

    in_maps = []
    gmaps = []
    for core in range(N_CORES):
        m, gm = _build_core_inputs(h, idx, counts, starts, wins, core, shared)
        in_maps.append(m)
        gmaps.append(gm)

    _cached["in_maps"] = in_maps
    if "nc" not in _cached:
        _cached["nc"] = _build_program()
    nc = _cached["nc"]

    from concourse import bass_utils
    res = bass_utils.run_bass_kernel_spmd(
        nc, in_maps, core_ids=list(range(N_CORES)))

    out = np.zeros((NUM_GRAPHS, NUM_TASKS), dtype=np.float32)
    for core in range(N_CORES):
        o = res.results[core]["out"]           # [10, 640]
        for g, s in gmaps[core]:
            out[g] = o[:, s]
    return out
